# revision 7
# baseline (speedup 1.0000x reference)
"""Trainium2 Bass kernel for nn_Affinity (gnn_message_passing).

M[(a,b),(c,d)] = sum_{j,i} H2[a,j]H2[c,j] H1[b,i]H1[d,i] W[j,i] + diag(Mp).

Structure exploited:
 - Nonzero blocks (a,c) of M: a==c or (a,c) an edge of graph 2 -> "slots".
   626 slots total, balanced 9 bands/core across 8 cores (<=79 slots/core).
 - Within a block, support is graph-1 adjacency + diagonal: only 622 of 5184
   (b,d) positions can be nonzero, the SAME set for every slot.
 - Per-slot weights w_s[i] = sum_q Me[q,i] SELT[q,s] factor through
   ZS = Xsum^T SELT so the edge-affinity matrix Me is never materialized.
 - Block values OUT[s, f] = sum_i wt[i,s] KRS[i, f] where
   KRS[i,(b,d)] = H1[b,i]H1[d,i] is a host-built 0/1 Khatri-Rao table
   restricted to the support -> one 3-pass matmul chain with partition=slot,
   so results DMA straight out with no transpose/scatter.
 - diag(Mp) folds in as a 4th PSUM accumulation against a [I72|0] table.

All index-derived tables (incidence, SELT, KRS, OHSS, IDPAD) are host-built;
every floating-point op runs on device. Host assembly only places computed
values (and zeros) into the full [5184, 5184] output.
"""
import sys
sys.path.insert(0, '/opt/trn_rl_repo')
import numpy as np

N = 72
E = 288
D = 64
NC = 8
SPAD = 80          # padded slots per core (max observed 79)
NSUP_PAD = 640     # padded support columns (622 actual), 2 chunks of 320
CH = 320           # PSUM free-chunk width


def _split_waits(nc, limit=1):
    """This walrus build rejects instructions with >limit sem waits; move the
    excess onto same-engine NoOps inserted immediately before (same bb order =
    same engine program order, so semantics are preserved)."""
    import concourse.mybir as mybir
    for f in nc.m.functions:
        for bb in f.blocks:
            new_insts = []
            for inst in bb.instructions:
                si = inst.sync_info
                waits = list(si.on_wait) if si and si.on_wait else []
                if len(waits) > limit:
                    extra, keep = waits[:-limit], waits[-limit:]
                    for i in range(0, len(extra), limit):
                        nop = mybir.InstNoOp(
                            name=nc.get_next_instruction_name(),
                            engine=inst.engine, ins=[], outs=[],
                            sync_info=mybir.SyncInfo(
                                on_wait=extra[i:i + limit], on_update=[]),
                        )
                        nc.register_instruction(nop)
                        new_insts.append(nop)
                    si.on_wait = keep
                new_insts.append(inst)
            bb.instructions[:] = new_insts


def _incidence(src, dst):
    H = np.zeros((N, E), np.float32)
    H[src, np.arange(E)] = 1.0
    H[dst, np.arange(E)] = 1.0
    return H


def _neighbors(src, dst):
    nbrs = [set() for _ in range(N)]
    for s, d in zip(src, dst):
        nbrs[int(s)].add(int(d))
        nbrs[int(d)].add(int(s))
    return nbrs


def _plan_assignment(nbrs2):
    """9 bands per core, greedily balancing slot count (1 + deg per band)."""
    deg = [len(x) for x in nbrs2]
    order = sorted(range(N), key=lambda a: -deg[a])
    cores = [[] for _ in range(NC)]
    loads = [0] * NC
    for a in order:
        c = min((c for c in range(NC) if len(cores[c]) < 9),
                key=lambda c: loads[c])
        cores[c].append(a)
        loads[c] += 1 + deg[a]
    assert max(loads) <= SPAD
    return cores


_FIELDS = [("f1", 72, D), ("f2", 72, D), ("h1", 72, E), ("s2", 72, E),
           ("d2m", 72, E), ("l1t", D, D), ("l2t", D, D), ("u1t", D, 72),
           ("u2t", D, 72), ("ohss", 72, SPAD), ("idpad", 72, CH),
           ("selt0", 96, SPAD), ("selt1", 96, SPAD), ("selt2", 96, SPAD)]
# wave-1 needs fields up through u2t; the rest (ohss/idpad/selt*) load late
_EARLY_COLS = 2 * D + 3 * E + 2 * D + 2 * 72


def _pk_offs():
    offs, pw = {}, 0
    for nm, r, w in _FIELDS:
        offs[nm] = pw
        pw += w
    return offs, pw


def _build_nc():
    import concourse.bass as bass
    import concourse.mybir as mybir
    import concourse.tile as tile

    F32 = mybir.dt.float32
    BF16 = mybir.dt.bfloat16

    offs, PW = _pk_offs()

    nc = bass.Bass()
    packb_d = nc.declare_dram_parameter("PACKB", [96, PW], BF16, isOutput=False)
    krs_d = nc.declare_dram_parameter("KRSD", [96, 3 * NSUP_PAD], BF16,
                                      isOutput=False)
    out_d = nc.declare_dram_parameter("OUT", [SPAD, NSUP_PAD], F32,
                                      isOutput=True)

    with tile.TileContext(nc) as tc:
        with tc.tile_pool(name="cst", bufs=1) as cst, \
             tc.tile_pool(name="ps", bufs=4, space="PSUM") as ps, \
             tc.tile_pool(name="psb", bufs=4, space="PSUM") as psb:

            pkb = cst.tile([96, PW], BF16)
            krs = cst.tile([96, 3 * NSUP_PAD], BF16)
            # early fields (wave-1 inputs) first at full bandwidth; the late
            # fields and KRS stream behind / in parallel while wave 1 runs
            ecols = _EARLY_COLS
            nc.sync.dma_start(out=pkb[:, 0:ecols], in_=packb_d[:, 0:ecols])
            nc.gpsimd.dma_start(out=pkb[:, ecols:PW], in_=packb_d[:, ecols:PW])
            nc.sync.dma_start(out=krs[:], in_=krs_d[:])

            def fld(nm):
                r, w = next((r, w) for n, r, w in _FIELDS if n == nm)
                return pkb[0:r, offs[nm]:offs[nm] + w]

            f1, f2, h1 = fld("f1"), fld("f2"), fld("h1")
            s2, d2m = fld("s2"), fld("d2m")
            l1t, l2t = fld("l1t"), fld("l2t")
            u1t, u2t = fld("u1t"), fld("u2t")
            ohss, idpad = fld("ohss"), fld("idpad")
            selt = [fld(f"selt{q}") for q in range(3)]

            # relu(lamda^T) tiles (DVE, straight from the input pack)
            r1t = cst.tile([D, D], BF16, tag="r1t")
            nc.vector.tensor_relu(out=r1t[:], in_=l1t)
            r2t = cst.tile([D, D], BF16, tag="r2t")
            nc.vector.tensor_relu(out=r2t[:], in_=l2t)

            # --- PE wave 1: everything that depends only on the input pack.
            # fs = F2^T S2, fdt = F2^T D2M  [D, E]
            fs_p = ps.tile([D, E], F32, tag="mm")
            nc.tensor.matmul(out=fs_p[:], lhsT=f2, rhs=s2, start=True, stop=True)
            fd_p = ps.tile([D, E], F32, tag="mm")
            nc.tensor.matmul(out=fd_p[:], lhsT=f2, rhs=d2m, start=True, stop=True)
            # Xsum chunks: z1tT[q][p, d] = sum_n H1[n, 96q+p] F1[n, d],
            # three 64-wide regions of one PSUM tile -> one copy.
            zp = ps.tile([96, 3 * D], F32, tag="mm")
            for q in range(3):
                nc.tensor.matmul(out=zp[:, D * q:D * (q + 1)],
                                 lhsT=h1[:, 96 * q:96 * (q + 1)], rhs=f1,
                                 start=True, stop=True)
            # Mp = U1 U2^T  [72, 72]
            mp_p = ps.tile([72, 72], F32, tag="mm")
            nc.tensor.matmul(out=mp_p[:], lhsT=u1t, rhs=u2t, start=True, stop=True)

            fsc = cst.tile([D, E], BF16, tag="fsc")
            nc.scalar.copy(out=fsc[:], in_=fs_p[:])
            fdc = cst.tile([D, E], BF16, tag="fdc")
            nc.vector.tensor_copy(out=fdc[:], in_=fd_p[:])
            z1c = cst.tile([96, 3 * D], BF16, tag="z1c")
            nc.vector.tensor_copy(out=z1c[:], in_=zp[:])
            mpc = cst.tile([72, 72], BF16, tag="mpc")
            nc.scalar.copy(out=mpc[:], in_=mp_p[:])

            # --- PE wave 2.
            # vvT chunks: vvT[96q+p, d] = (Ys relu(L1)^T + Yd relu(L2)^T)
            vvt_p = ps.tile([96, 3 * D], F32, tag="mm")
            for q in range(3):
                nc.tensor.matmul(out=vvt_p[:, D * q:D * (q + 1)],
                                 lhsT=fsc[:, 96 * q:96 * (q + 1)], rhs=r1t[:],
                                 start=True, stop=False)
                nc.tensor.matmul(out=vvt_p[:, D * q:D * (q + 1)],
                                 lhsT=fdc[:, 96 * q:96 * (q + 1)], rhs=r2t[:],
                                 start=False, stop=True)
            # ZS[d, s] = sum_q Xsum[q, d] SELT[q, s]  [D, SPAD]
            zs_p = ps.tile([D, SPAD], F32, tag="mm")
            for q in range(3):
                nc.tensor.matmul(out=zs_p[:], lhsT=z1c[:, D * q:D * (q + 1)],
                                 rhs=selt[q], start=(q == 0), stop=(q == 2))
            # mpx[p, s] = Mp[a_s, p] for diag slots (cols of OHSS), else 0
            mpx_p = ps.tile([72, SPAD], F32, tag="mm")
            nc.tensor.matmul(out=mpx_p[:], lhsT=mpc[:], rhs=ohss,
                             start=True, stop=True)

            vvtc = cst.tile([96, 3 * D], BF16, tag="vvtc")
            nc.vector.tensor_copy(out=vvtc[:], in_=vvt_p[:])
            zsc = cst.tile([D, SPAD], BF16, tag="zsc")
            nc.scalar.copy(out=zsc[:], in_=zs_p[:])
            mpxc = cst.tile([72, SPAD], BF16, tag="mpxc")
            nc.scalar.copy(out=mpxc[:], in_=mpx_p[:])

            # --- PE wave 3: VK[d, f] = sum_i vvT[i, d] KRS[i, f]
            vkc = []
            for ch in range(2):
                vk_p = psb.tile([D, CH], F32, tag="bb")
                for q in range(3):
                    nc.tensor.matmul(
                        out=vk_p[:], lhsT=vvtc[:, D * q:D * (q + 1)],
                        rhs=krs[0:96, q * NSUP_PAD + ch * CH:
                                q * NSUP_PAD + (ch + 1) * CH],
                        start=(q == 0), stop=(q == 2))
                vc = cst.tile([D, CH], BF16, tag=f"vkc{ch}")
                if ch == 0:
                    nc.vector.tensor_copy(out=vc[:], in_=vk_p[:])
                else:
                    nc.scalar.copy(out=vc[:], in_=vk_p[:])
                vkc.append(vc)

            # --- PE wave 4: OUT[s, f] = sum_d ZS[d, s] VK[d, f] (+ Mp diag)
            staged = cst.tile([SPAD, NSUP_PAD], F32)
            for ch in range(2):
                bp = psb.tile([SPAD, CH], F32, tag="bb")
                nc.tensor.matmul(out=bp[:], lhsT=zsc[:], rhs=vkc[ch][:],
                                 start=True, stop=(ch == 1))
                if ch == 0:
                    # diag(Mp) add: rhs = [I72 | 0], lhsT = mpx
                    nc.tensor.matmul(out=bp[:], lhsT=mpxc[:], rhs=idpad,
                                     start=False, stop=True)
                if ch == 0:
                    nc.vector.tensor_copy(
                        out=staged[:, ch * CH:(ch + 1) * CH], in_=bp[:])
                else:
                    nc.scalar.copy(
                        out=staged[:, ch * CH:(ch + 1) * CH], in_=bp[:])
                eng = nc.sync if ch == 0 else nc.scalar
                eng.dma_start(out=out_d[:, ch * CH:(ch + 1) * CH],
                              in_=staged[:, ch * CH:(ch + 1) * CH])

    _split_waits(nc)
    return nc


def _prepare(inputs):
    import ml_dtypes
    ins = {k: np.asarray(v) for k, v in inputs.items()}
    F1 = ins["F1"].astype(np.float32)
    F2 = ins["F2"].astype(np.float32)
    U1 = ins["U1"].astype(np.float32)
    U2 = ins["U2"].astype(np.float32)
    l1 = ins["lamda1"].astype(np.float32)
    l2 = ins["lamda2"].astype(np.float32)
    src1 = ins["src1"].astype(np.int64)
    dst1 = ins["dst1"].astype(np.int64)
    src2 = ins["src2"].astype(np.int64)
    dst2 = ins["dst2"].astype(np.int64)

    H1 = _incidence(src1, dst1)
    H2 = _incidence(src2, dst2)
    S2 = np.zeros((N, E), np.float32)
    S2[src2, np.arange(E)] = 1.0
    D2M = np.zeros((N, E), np.float32)
    D2M[dst2, np.arange(E)] = 1.0

    nbrs2 = _neighbors(src2, dst2)
    nbrs1 = _neighbors(src1, dst1)
    cores = _plan_assignment(nbrs2)

    # support: diag first (col b = (b,b)), then off-diag adjacency pairs
    supp = [(b, b) for b in range(N)]
    for b in range(N):
        for d in sorted(nbrs1[b]):
            supp.append((b, d))
    nsup = len(supp)
    assert nsup <= NSUP_PAD
    KRS = np.zeros((E, NSUP_PAD), np.float32)
    for f, (b, d) in enumerate(supp):
        KRS[:, f] = H1[b] * H1[d]
    KRSD = np.zeros((96, 3 * NSUP_PAD), ml_dtypes.bfloat16)
    for ic in range(3):
        KRSD[:, ic * NSUP_PAD:(ic + 1) * NSUP_PAD] = \
            KRS[96 * ic:96 * (ic + 1)].astype(ml_dtypes.bfloat16)

    offs, PW = _pk_offs()
    base = np.zeros((96, PW), ml_dtypes.bfloat16)

    def put(arr, nm, r=None):
        rr, w = arr.shape
        base[0:rr, offs[nm]:offs[nm] + w] = arr.astype(ml_dtypes.bfloat16)
    put(F1, "f1")
    put(F2, "f2")
    put(H1, "h1")
    put(S2, "s2")
    put(D2M, "d2m")
    put(np.ascontiguousarray(l1.T), "l1t")
    put(np.ascontiguousarray(l2.T), "l2t")
    put(np.ascontiguousarray(U1.T), "u1t")
    put(np.ascontiguousarray(U2.T), "u2t")
    IDPAD = np.zeros((72, CH), np.float32)
    IDPAD[np.arange(72), np.arange(72)] = 1.0
    put(IDPAD, "idpad")

    in_maps = []
    slot_maps = []
    for c in range(NC):
        slots = []
        for a in cores[c]:
            slots.append((a, a))
            for cc in sorted(nbrs2[a]):
                slots.append((a, cc))
        SELT = np.zeros((E, SPAD), np.float32)
        OHSS = np.zeros((72, SPAD), np.float32)
        di = 0
        for s_i, (a, cc) in enumerate(slots):
            SELT[:, s_i] = H2[a] * H2[cc]
            if a == cc:
                OHSS[a, s_i] = 1.0
                di += 1
        pack = base.copy()
        pack[0:72, offs["ohss"]:offs["ohss"] + SPAD] = \
            OHSS.astype(ml_dtypes.bfloat16)
        for q in range(3):
            pack[0:96, offs[f"selt{q}"]:offs[f"selt{q}"] + SPAD] = \
                SELT[96 * q:96 * (q + 1)].astype(ml_dtypes.bfloat16)
        in_maps.append({"PACKB": pack, "KRSD": KRSD})
        slot_maps.append(slots)
    supp_off = np.array([b * (N * N) + d for b, d in supp], np.int64)
    return in_maps, slot_maps, supp_off


_CACHE = {}


def kernel(**inputs):
    from concourse.bass_utils import run_bass_kernel_spmd

    in_maps, slot_maps, supp_off = _prepare(inputs)
    nc = _CACHE.get("nc")
    if nc is None:
        nc = _build_nc()
        _CACHE["nc"] = nc
    res = run_bass_kernel_spmd(nc, in_maps, list(range(NC)))
    nsup = len(supp_off)
    M = np.zeros((N * N, N * N), np.float32)
    for c in range(NC):
        out = res.results[c]["OUT"]
        slots = slot_maps[c]
        bases = np.array([a * (N * N * N) + cc * N for a, cc in slots],
                         np.int64)
        M.flat[bases[:, None] + supp_off[None, :]] = out[:len(slots), :nsup]
    return M


# revision 8
# speedup vs baseline: 1.0899x; 1.0899x over previous
"""Trainium2 Bass kernel for nn_Affinity (gnn_message_passing).

M[(a,b),(c,d)] = sum_{j,i} H2[a,j]H2[c,j] H1[b,i]H1[d,i] W[j,i] + diag(Mp).

Structure exploited:
 - Nonzero blocks (a,c) of M: a==c or (a,c) an edge of graph 2 -> "slots".
   626 slots total, balanced 9 bands/core across 8 cores (<=79 slots/core).
 - Within a block, only graph-1-adjacent (b,d) positions (and the diagonal)
   are nonzero; blocks are symmetric in (b,d), so each block is fully
   described by 72 diagonal values + one value per unique adjacent pair
   (275 of them) -> device output is [80 slots, 352] per core.
 - Per-slot weights factor through ZS = Xsum^T SELT (the edge-affinity
   matrix Me is never materialized), and the block values are
   OUTD = ZS^T (vv H1^T), OUTO = ZS^T (vv R) where R merges multi-edges
   of graph 1 into unique pairs. diag(Mp) folds in as an extra PSUM
   accumulation against an identity table.

All index-derived tables (incidence, SELT, R, H1T, OHSS, I72) are
host-built 0/1 matrices; every floating-point op runs on device. Host
assembly only places computed values (and zeros) into the [5184, 5184]
output.
"""
import sys
sys.path.insert(0, '/opt/trn_rl_repo')
import numpy as np

N = 72
E = 288
D = 64
NC = 8


def _split_waits(nc, limit=1):
    """This walrus build rejects instructions with >limit sem waits; move the
    excess onto same-engine NoOps inserted immediately before (same bb order =
    same engine program order, so semantics are preserved)."""
    import concourse.mybir as mybir
    for f in nc.m.functions:
        for bb in f.blocks:
            new_insts = []
            for inst in bb.instructions:
                si = inst.sync_info
                waits = list(si.on_wait) if si and si.on_wait else []
                if len(waits) > limit:
                    extra, keep = waits[:-limit], waits[-limit:]
                    for i in range(0, len(extra), limit):
                        nop = mybir.InstNoOp(
                            name=nc.get_next_instruction_name(),
                            engine=inst.engine, ins=[], outs=[],
                            sync_info=mybir.SyncInfo(
                                on_wait=extra[i:i + limit], on_update=[]),
                        )
                        nc.register_instruction(nop)
                        new_insts.append(nop)
                    si.on_wait = keep
                new_insts.append(inst)
            bb.instructions[:] = new_insts


def _incidence(src, dst):
    H = np.zeros((N, E), np.float32)
    H[src, np.arange(E)] = 1.0
    H[dst, np.arange(E)] = 1.0
    return H


def _neighbors(src, dst):
    nbrs = [set() for _ in range(N)]
    for s, d in zip(src, dst):
        nbrs[int(s)].add(int(d))
        nbrs[int(d)].add(int(s))
    return nbrs


def _plan_assignment(nbrs2, spad):
    """9 bands per core, greedily balancing slot count (1 + deg per band)."""
    deg = [len(x) for x in nbrs2]
    order = sorted(range(N), key=lambda a: -deg[a])
    cores = [[] for _ in range(NC)]
    loads = [0] * NC
    for a in order:
        c = min((c for c in range(NC) if len(cores[c]) < 9),
                key=lambda c: loads[c])
        cores[c].append(a)
        loads[c] += 1 + deg[a]
    assert max(loads) <= spad
    return cores


def _build_nc(SPAD, NUPAD):
    import concourse.bass as bass
    import concourse.mybir as mybir
    import concourse.tile as tile

    F32 = mybir.dt.float32
    BF16 = mybir.dt.bfloat16
    OUTW = 72 + NUPAD

    nc = bass.Bass()
    # pack widths: [72-row early] [64-row early] [72-row late] [96-row late]
    W72E = 2 * D + 3 * E
    W64E = 2 * D + 2 * 72
    W72L = SPAD + 72
    W96L = 3 * SPAD + 3 * NUPAD + 3 * 72
    pk72e_d = nc.declare_dram_parameter("PK72E", [72, W72E], BF16, isOutput=False)
    pk64e_d = nc.declare_dram_parameter("PK64E", [64, W64E], BF16, isOutput=False)
    pk72l_d = nc.declare_dram_parameter("PK72L", [72, W72L], BF16, isOutput=False)
    pk96l_d = nc.declare_dram_parameter("PK96L", [96, W96L], BF16, isOutput=False)
    out_d = nc.declare_dram_parameter("OUT", [SPAD, OUTW], F32, isOutput=True)

    with tile.TileContext(nc) as tc:
        with tc.tile_pool(name="cst", bufs=1) as cst, \
             tc.tile_pool(name="ps", bufs=4, space="PSUM") as ps, \
             tc.tile_pool(name="psb", bufs=4, space="PSUM") as psb:

            p72e = cst.tile([72, W72E], BF16)
            p64e = cst.tile([64, W64E], BF16)
            p72l = cst.tile([72, W72L], BF16)
            p96l = cst.tile([96, W96L], BF16)
            nc.sync.dma_start(out=p72e[:], in_=pk72e_d[:])
            nc.scalar.dma_start(out=p64e[:], in_=pk64e_d[:])
            nc.gpsimd.dma_start(out=p96l[:], in_=pk96l_d[:])
            nc.scalar.dma_start(out=p72l[:], in_=pk72l_d[:])

            f1 = p72e[:, 0:D]
            f2 = p72e[:, D:2 * D]
            h1 = p72e[:, 2 * D:2 * D + E]
            s2 = p72e[:, 2 * D + E:2 * D + 2 * E]
            d2m = p72e[:, 2 * D + 2 * E:2 * D + 3 * E]
            l1t = p64e[:, 0:D]
            l2t = p64e[:, D:2 * D]
            u1t = p64e[:, 2 * D:2 * D + 72]
            u2t = p64e[:, 2 * D + 72:2 * D + 144]
            ohss = p72l[:, 0:SPAD]
            id72 = p72l[:, SPAD:SPAD + 72]
            selt = [p96l[:, SPAD * q:SPAD * (q + 1)] for q in range(3)]
            rr = [p96l[:, 3 * SPAD + NUPAD * q:3 * SPAD + NUPAD * (q + 1)]
                  for q in range(3)]
            h1t = [p96l[:, 3 * SPAD + 3 * NUPAD + 72 * q:
                        3 * SPAD + 3 * NUPAD + 72 * (q + 1)] for q in range(3)]

            # relu(lamda^T) tiles (DVE, straight from the input pack)
            r1t = cst.tile([D, D], BF16, tag="r1t")
            nc.vector.tensor_relu(out=r1t[:], in_=l1t)
            r2t = cst.tile([D, D], BF16, tag="r2t")
            nc.vector.tensor_relu(out=r2t[:], in_=l2t)

            # --- PE wave 1: everything that depends only on the early packs.
            fs_p = ps.tile([D, E], F32, tag="mm")
            nc.tensor.matmul(out=fs_p[:], lhsT=f2, rhs=s2, start=True, stop=True)
            fd_p = ps.tile([D, E], F32, tag="mm")
            nc.tensor.matmul(out=fd_p[:], lhsT=f2, rhs=d2m, start=True, stop=True)
            # Xsum chunks: three 64-wide regions of one PSUM tile -> one copy
            zp = ps.tile([96, 3 * D], F32, tag="mm")
            for q in range(3):
                nc.tensor.matmul(out=zp[:, D * q:D * (q + 1)],
                                 lhsT=h1[:, 96 * q:96 * (q + 1)], rhs=f1,
                                 start=True, stop=True)
            mp_p = ps.tile([72, 72], F32, tag="mm")
            nc.tensor.matmul(out=mp_p[:], lhsT=u1t, rhs=u2t, start=True, stop=True)

            fsc = cst.tile([D, E], BF16, tag="fsc")
            nc.scalar.copy(out=fsc[:], in_=fs_p[:])
            fdc = cst.tile([D, E], BF16, tag="fdc")
            nc.vector.tensor_copy(out=fdc[:], in_=fd_p[:])
            z1c = cst.tile([96, 3 * D], BF16, tag="z1c")
            nc.vector.tensor_copy(out=z1c[:], in_=zp[:])
            mpc = cst.tile([72, 72], BF16, tag="mpc")
            nc.scalar.copy(out=mpc[:], in_=mp_p[:])

            # --- PE wave 2.
            # vvT chunks: vvT[96q+p, d] = (Ys relu(L1)^T + Yd relu(L2)^T)
            vvt_p = ps.tile([96, 3 * D], F32, tag="mm")
            for q in range(3):
                nc.tensor.matmul(out=vvt_p[:, D * q:D * (q + 1)],
                                 lhsT=fsc[:, 96 * q:96 * (q + 1)], rhs=r1t[:],
                                 start=True, stop=False)
                nc.tensor.matmul(out=vvt_p[:, D * q:D * (q + 1)],
                                 lhsT=fdc[:, 96 * q:96 * (q + 1)], rhs=r2t[:],
                                 start=False, stop=True)
            # ZS[d, s] = sum_q Xsum[q, d] SELT[q, s]  [D, SPAD]
            zs_p = ps.tile([D, SPAD], F32, tag="mm")
            for q in range(3):
                nc.tensor.matmul(out=zs_p[:], lhsT=z1c[:, D * q:D * (q + 1)],
                                 rhs=selt[q], start=(q == 0), stop=(q == 2))
            # mpx[p, s] = Mp[a_s, p] for diag slots (cols of OHSS), else 0
            mpx_p = ps.tile([72, SPAD], F32, tag="mm")
            nc.tensor.matmul(out=mpx_p[:], lhsT=mpc[:], rhs=ohss,
                             start=True, stop=True)

            vvtc = cst.tile([96, 3 * D], BF16, tag="vvtc")
            nc.vector.tensor_copy(out=vvtc[:], in_=vvt_p[:])
            zsc = cst.tile([D, SPAD], BF16, tag="zsc")
            nc.scalar.copy(out=zsc[:], in_=zs_p[:])
            mpxc = cst.tile([72, SPAD], BF16, tag="mpxc")
            nc.scalar.copy(out=mpxc[:], in_=mpx_p[:])

            # --- PE wave 3: vkd = vv H1^T [D, 72]; vvm = vv R [D, NUPAD]
            vkd_p = psb.tile([D, 72], F32, tag="bb")
            for q in range(3):
                nc.tensor.matmul(out=vkd_p[:], lhsT=vvtc[:, D * q:D * (q + 1)],
                                 rhs=h1t[q], start=(q == 0), stop=(q == 2))
            vkdc = cst.tile([D, 72], BF16, tag="vkdc")
            nc.scalar.copy(out=vkdc[:], in_=vkd_p[:])
            vvm_p = psb.tile([D, NUPAD], F32, tag="bb")
            for q in range(3):
                nc.tensor.matmul(out=vvm_p[:], lhsT=vvtc[:, D * q:D * (q + 1)],
                                 rhs=rr[q], start=(q == 0), stop=(q == 2))
            vvmc = cst.tile([D, NUPAD], BF16, tag="vvmc")
            nc.vector.tensor_copy(out=vvmc[:], in_=vvm_p[:])

            # --- PE wave 4: OUT[s, :72] = ZS^T vkd + Mp diag;
            #                OUT[s, 72:] = ZS^T vvm
            staged = cst.tile([SPAD, OUTW], F32)
            bp = psb.tile([SPAD, OUTW], F32, tag="bb")
            nc.tensor.matmul(out=bp[:, 0:72], lhsT=zsc[:], rhs=vkdc[:],
                             start=True, stop=False)
            nc.tensor.matmul(out=bp[:, 0:72], lhsT=mpxc[:], rhs=id72,
                             start=False, stop=True)
            nc.tensor.matmul(out=bp[:, 72:OUTW], lhsT=zsc[:], rhs=vvmc[:],
                             start=True, stop=True)
            half = (OUTW // 2) // 4 * 4
            nc.vector.tensor_copy(out=staged[:, 0:half], in_=bp[:, 0:half])
            nc.scalar.copy(out=staged[:, half:OUTW], in_=bp[:, half:OUTW])
            nc.sync.dma_start(out=out_d[:], in_=staged[:])

    _split_waits(nc)
    return nc


def _prepare(inputs):
    import ml_dtypes
    ins = {k: np.asarray(v) for k, v in inputs.items()}
    F1 = ins["F1"].astype(np.float32)
    F2 = ins["F2"].astype(np.float32)
    U1 = ins["U1"].astype(np.float32)
    U2 = ins["U2"].astype(np.float32)
    l1 = ins["lamda1"].astype(np.float32)
    l2 = ins["lamda2"].astype(np.float32)
    src1 = ins["src1"].astype(np.int64)
    dst1 = ins["dst1"].astype(np.int64)
    src2 = ins["src2"].astype(np.int64)
    dst2 = ins["dst2"].astype(np.int64)

    H1 = _incidence(src1, dst1)
    H2 = _incidence(src2, dst2)
    S2 = np.zeros((N, E), np.float32)
    S2[src2, np.arange(E)] = 1.0
    D2M = np.zeros((N, E), np.float32)
    D2M[dst2, np.arange(E)] = 1.0

    nbrs2 = _neighbors(src2, dst2)
    # unique unordered adjacent pairs of graph 1 + multi-edge merge R
    pairs = {}
    for i, (s, d) in enumerate(zip(src1, dst1)):
        pairs.setdefault((int(s), int(d)), []).append(i)
    plist = sorted(pairs)
    NU = len(plist)
    NUPAD = (NU + 7) // 8 * 8
    deg2 = [1 + len(x) for x in nbrs2]
    SPAD = max(80, (max(deg2) + sum(deg2) // NC + 7) // 8 * 8)
    cores = _plan_assignment(nbrs2, SPAD)

    R = np.zeros((E, NUPAD), np.float32)
    for u, key in enumerate(plist):
        for i in pairs[key]:
            R[i, u] = 1.0

    bf = ml_dtypes.bfloat16
    W72E = 2 * D + 3 * E
    W64E = 2 * D + 2 * 72
    W72L = SPAD + 72
    W96L = 3 * SPAD + 3 * NUPAD + 3 * 72
    PK72E = np.zeros((72, W72E), bf)
    PK72E[:, 0:D] = F1.astype(bf)
    PK72E[:, D:2 * D] = F2.astype(bf)
    PK72E[:, 2 * D:2 * D + E] = H1.astype(bf)
    PK72E[:, 2 * D + E:2 * D + 2 * E] = S2.astype(bf)
    PK72E[:, 2 * D + 2 * E:2 * D + 3 * E] = D2M.astype(bf)
    PK64E = np.zeros((64, W64E), bf)
    PK64E[:, 0:D] = l1.T.astype(bf)
    PK64E[:, D:2 * D] = l2.T.astype(bf)
    PK64E[:, 2 * D:2 * D + 72] = U1.T.astype(bf)
    PK64E[:, 2 * D + 72:2 * D + 144] = U2.T.astype(bf)
    PK96L = np.zeros((96, W96L), bf)
    for q in range(3):
        PK96L[:, 3 * SPAD + NUPAD * q:3 * SPAD + NUPAD * (q + 1)] = \
            R[96 * q:96 * (q + 1)].astype(bf)
        PK96L[:, 3 * SPAD + 3 * NUPAD + 72 * q:
              3 * SPAD + 3 * NUPAD + 72 * (q + 1)] = \
            H1.T[96 * q:96 * (q + 1)].astype(bf)
    ID72 = np.eye(72, dtype=np.float32)

    in_maps = []
    slot_maps = []
    for c in range(NC):
        slots = []
        for a in cores[c]:
            slots.append((a, a))
            for cc in sorted(nbrs2[a]):
                slots.append((a, cc))
        SELT = np.zeros((E, SPAD), np.float32)
        OHSS = np.zeros((72, SPAD), np.float32)
        for s_i, (a, cc) in enumerate(slots):
            SELT[:, s_i] = H2[a] * H2[cc]
            if a == cc:
                OHSS[a, s_i] = 1.0
        PK72L = np.zeros((72, W72L), bf)
        PK72L[:, 0:SPAD] = OHSS.astype(bf)
        PK72L[:, SPAD:SPAD + 72] = ID72.astype(bf)
        pk96 = PK96L.copy()
        for q in range(3):
            pk96[:, SPAD * q:SPAD * (q + 1)] = \
                SELT[96 * q:96 * (q + 1)].astype(bf)
        in_maps.append({"PK72E": PK72E, "PK64E": PK64E,
                        "PK72L": PK72L, "PK96L": pk96})
        slot_maps.append(slots)

    # host assembly maps: value columns + flat offsets within a block
    col_idx = np.concatenate([np.arange(72),
                              np.repeat(72 + np.arange(NU), 2)])
    offs = [b * (N * N + 1) for b in range(72)]
    for (b, d) in plist:
        offs.append(b * N * N + d)
        offs.append(d * N * N + b)
    offs_all = np.array(offs, np.int64)
    return in_maps, slot_maps, col_idx, offs_all, SPAD, NUPAD


_CACHE = {}


def kernel(**inputs):
    from concourse.bass_utils import run_bass_kernel_spmd

    in_maps, slot_maps, col_idx, offs_all, SPAD, NUPAD = _prepare(inputs)
    key = (SPAD, NUPAD)
    nc = _CACHE.get(key)
    if nc is None:
        nc = _build_nc(SPAD, NUPAD)
        _CACHE[key] = nc
    res = run_bass_kernel_spmd(nc, in_maps, list(range(NC)))
    M = np.zeros((N * N, N * N), np.float32)
    for c in range(NC):
        out = res.results[c]["OUT"]
        slots = slot_maps[c]
        bases = np.array([a * (N * N * N) + cc * N for a, cc in slots],
                         np.int64)
        M.flat[bases[:, None] + offs_all[None, :]] = \
            out[:len(slots)][:, col_idx]
    return M


# revision 9
# speedup vs baseline: 1.1850x; 1.0872x over previous
"""Trainium2 Bass kernel for nn_Affinity (gnn_message_passing).

M[(a,b),(c,d)] = sum_{j,i} H2[a,j]H2[c,j] H1[b,i]H1[d,i] W[j,i] + diag(Mp).

Structure exploited:
 - Nonzero blocks (a,c) of M: a==c or (a,c) an edge of graph 2 -> "slots".
   626 slots total, balanced 9 bands/core across 8 cores (<=79 slots/core).
 - Within a block, only graph-1-adjacent (b,d) positions (and the diagonal)
   are nonzero; blocks are symmetric in (b,d), so each block is fully
   described by 72 diagonal values + one value per unique adjacent pair
   (275 of them) -> device output is [80 slots, 352] per core.
 - Per-slot weights factor through ZS = Xsum^T SELT (the edge-affinity
   matrix Me is never materialized), and the block values are
   OUTD = ZS^T (vv H1^T), OUTO = ZS^T (vv R) where R merges multi-edges
   of graph 1 into unique pairs. diag(Mp) folds in as an extra PSUM
   accumulation against an identity table.

All index-derived tables (incidence, SELT, R, H1T, OHSS, I72) are
host-built 0/1 matrices; every floating-point op runs on device. Host
assembly only places computed values (and zeros) into the [5184, 5184]
output.
"""
import sys
sys.path.insert(0, '/opt/trn_rl_repo')
import numpy as np

N = 72
E = 288
D = 64
NC = 8


def _split_waits(nc, limit=1):
    """This walrus build rejects instructions with >limit sem waits; move the
    excess onto same-engine NoOps inserted immediately before (same bb order =
    same engine program order, so semantics are preserved)."""
    import concourse.mybir as mybir
    for f in nc.m.functions:
        for bb in f.blocks:
            new_insts = []
            for inst in bb.instructions:
                si = inst.sync_info
                waits = list(si.on_wait) if si and si.on_wait else []
                if len(waits) > limit:
                    extra, keep = waits[:-limit], waits[-limit:]
                    for i in range(0, len(extra), limit):
                        nop = mybir.InstNoOp(
                            name=nc.get_next_instruction_name(),
                            engine=inst.engine, ins=[], outs=[],
                            sync_info=mybir.SyncInfo(
                                on_wait=extra[i:i + limit], on_update=[]),
                        )
                        nc.register_instruction(nop)
                        new_insts.append(nop)
                    si.on_wait = keep
                new_insts.append(inst)
            bb.instructions[:] = new_insts


def _incidence(src, dst):
    H = np.zeros((N, E), np.float32)
    H[src, np.arange(E)] = 1.0
    H[dst, np.arange(E)] = 1.0
    return H


def _neighbors(src, dst):
    nbrs = [set() for _ in range(N)]
    for s, d in zip(src, dst):
        nbrs[int(s)].add(int(d))
        nbrs[int(d)].add(int(s))
    return nbrs


def _plan_assignment(nbrs2, spad):
    """9 bands per core, greedily balancing slot count (1 + deg per band)."""
    deg = [len(x) for x in nbrs2]
    order = sorted(range(N), key=lambda a: -deg[a])
    cores = [[] for _ in range(NC)]
    loads = [0] * NC
    for a in order:
        c = min((c for c in range(NC) if len(cores[c]) < 9),
                key=lambda c: loads[c])
        cores[c].append(a)
        loads[c] += 1 + deg[a]
    assert max(loads) <= spad
    return cores


def _build_nc(SPAD, NUPAD):
    import concourse.bass as bass
    import concourse.mybir as mybir
    import concourse.tile as tile

    F32 = mybir.dt.float32
    BF16 = mybir.dt.bfloat16
    OUTW = 72 + NUPAD

    nc = bass.Bass()
    # single 72-row pack: f1 f2 s2r d2r s2h d2h hs ohss id72
    W72 = 2 * D + 2 * NUPAD + 2 * 72 + 2 * SPAD + 72
    W64 = 2 * D + 2 * 72
    pk72_d = nc.declare_dram_parameter("PK72", [72, W72], BF16, isOutput=False)
    pk64_d = nc.declare_dram_parameter("PK64", [64, W64], BF16, isOutput=False)
    out_d = nc.declare_dram_parameter("OUT", [SPAD, OUTW], F32, isOutput=True)

    with tile.TileContext(nc) as tc:
        with tc.tile_pool(name="cst", bufs=1) as cst, \
             tc.tile_pool(name="ps", bufs=4, space="PSUM") as ps, \
             tc.tile_pool(name="psb", bufs=4, space="PSUM") as psb:

            p72 = cst.tile([72, W72], BF16)
            p64 = cst.tile([64, W64], BF16)
            nc.sync.dma_start(out=p72[:], in_=pk72_d[:])
            nc.scalar.dma_start(out=p64[:], in_=pk64_d[:])

            o = 0
            def nxt(w):
                nonlocal o
                o += w
                return o - w
            f1 = p72[:, nxt(D):o]
            f2 = p72[:, nxt(D):o]
            s2r = p72[:, nxt(NUPAD):o]
            d2r = p72[:, nxt(NUPAD):o]
            s2h = p72[:, nxt(72):o]
            d2h = p72[:, nxt(72):o]
            hs = p72[:, nxt(SPAD):o]
            ohss = p72[:, nxt(SPAD):o]
            id72 = p72[:, nxt(72):o]
            l1t = p64[:, 0:D]
            l2t = p64[:, D:2 * D]
            u1t = p64[:, 2 * D:2 * D + 72]
            u2t = p64[:, 2 * D + 72:2 * D + 144]

            # relu(lamda^T) tiles (DVE, straight from the input pack)
            r1t = cst.tile([D, D], BF16, tag="r1t")
            nc.vector.tensor_relu(out=r1t[:], in_=l1t)
            r2t = cst.tile([D, D], BF16, tag="r2t")
            nc.vector.tensor_relu(out=r2t[:], in_=l2t)

            # --- PE wave 1: contract F1/F2/U against host-precontracted
            # integer tables.
            fsm_p = ps.tile([D, NUPAD], F32, tag="mm")
            nc.tensor.matmul(out=fsm_p[:], lhsT=f2, rhs=s2r, start=True, stop=True)
            fdm_p = ps.tile([D, NUPAD], F32, tag="mm")
            nc.tensor.matmul(out=fdm_p[:], lhsT=f2, rhs=d2r, start=True, stop=True)
            fh_p = ps.tile([D, 144], F32, tag="mm")
            nc.tensor.matmul(out=fh_p[:, 0:72], lhsT=f2, rhs=s2h, start=True, stop=True)
            nc.tensor.matmul(out=fh_p[:, 72:144], lhsT=f2, rhs=d2h, start=True, stop=True)
            zs_p = ps.tile([D, SPAD], F32, tag="mm")
            nc.tensor.matmul(out=zs_p[:], lhsT=f1, rhs=hs, start=True, stop=True)
            mp_p = ps.tile([72, 72], F32, tag="mm")
            nc.tensor.matmul(out=mp_p[:], lhsT=u1t, rhs=u2t, start=True, stop=True)

            fsmc = cst.tile([D, NUPAD], BF16, tag="fsmc")
            nc.scalar.copy(out=fsmc[:], in_=fsm_p[:])
            fdmc = cst.tile([D, NUPAD], BF16, tag="fdmc")
            nc.vector.tensor_copy(out=fdmc[:], in_=fdm_p[:])
            fhc = cst.tile([D, 144], BF16, tag="fhc")
            nc.vector.tensor_copy(out=fhc[:], in_=fh_p[:])
            zsc = cst.tile([D, SPAD], BF16, tag="zsc")
            nc.scalar.copy(out=zsc[:], in_=zs_p[:])
            mpc = cst.tile([72, 72], BF16, tag="mpc")
            nc.scalar.copy(out=mpc[:], in_=mp_p[:])

            # --- PE wave 2: apply relu(lamda) blocks; mpx select.
            vvm_p = psb.tile([D, NUPAD], F32, tag="bb")
            nc.tensor.matmul(out=vvm_p[:], lhsT=r1t[:], rhs=fsmc[:],
                             start=True, stop=False)
            nc.tensor.matmul(out=vvm_p[:], lhsT=r2t[:], rhs=fdmc[:],
                             start=False, stop=True)
            vkd_p = psb.tile([D, 72], F32, tag="bb")
            nc.tensor.matmul(out=vkd_p[:], lhsT=r1t[:], rhs=fhc[:, 0:72],
                             start=True, stop=False)
            nc.tensor.matmul(out=vkd_p[:], lhsT=r2t[:], rhs=fhc[:, 72:144],
                             start=False, stop=True)
            mpx_p = ps.tile([72, SPAD], F32, tag="mm")
            nc.tensor.matmul(out=mpx_p[:], lhsT=mpc[:], rhs=ohss,
                             start=True, stop=True)

            vvmc = cst.tile([D, NUPAD], BF16, tag="vvmc")
            nc.vector.tensor_copy(out=vvmc[:], in_=vvm_p[:])
            vkdc = cst.tile([D, 72], BF16, tag="vkdc")
            nc.scalar.copy(out=vkdc[:], in_=vkd_p[:])
            mpxc = cst.tile([72, SPAD], BF16, tag="mpxc")
            nc.scalar.copy(out=mpxc[:], in_=mpx_p[:])

            # --- PE wave 3: OUT[s, :72] = ZS^T vkd + Mp diag;
            #                OUT[s, 72:] = ZS^T vvm
            staged = cst.tile([SPAD, OUTW], F32)
            bp = psb.tile([SPAD, OUTW], F32, tag="bb")
            nc.tensor.matmul(out=bp[:, 72:OUTW], lhsT=zsc[:], rhs=vvmc[:],
                             start=True, stop=True)
            nc.tensor.matmul(out=bp[:, 0:72], lhsT=zsc[:], rhs=vkdc[:],
                             start=True, stop=False)
            nc.tensor.matmul(out=bp[:, 0:72], lhsT=mpxc[:], rhs=id72,
                             start=False, stop=True)
            half = (OUTW // 2) // 4 * 4
            nc.scalar.copy(out=staged[:, half:OUTW], in_=bp[:, half:OUTW])
            nc.vector.tensor_copy(out=staged[:, 0:half], in_=bp[:, 0:half])
            nc.sync.dma_start(out=out_d[:], in_=staged[:])

    _split_waits(nc)
    return nc


def _prepare(inputs):
    import ml_dtypes
    ins = {k: np.asarray(v) for k, v in inputs.items()}
    F1 = ins["F1"].astype(np.float32)
    F2 = ins["F2"].astype(np.float32)
    U1 = ins["U1"].astype(np.float32)
    U2 = ins["U2"].astype(np.float32)
    l1 = ins["lamda1"].astype(np.float32)
    l2 = ins["lamda2"].astype(np.float32)
    src1 = ins["src1"].astype(np.int64)
    dst1 = ins["dst1"].astype(np.int64)
    src2 = ins["src2"].astype(np.int64)
    dst2 = ins["dst2"].astype(np.int64)

    H1 = _incidence(src1, dst1)
    H2 = _incidence(src2, dst2)
    S2 = np.zeros((N, E), np.float32)
    S2[src2, np.arange(E)] = 1.0
    D2M = np.zeros((N, E), np.float32)
    D2M[dst2, np.arange(E)] = 1.0

    nbrs2 = _neighbors(src2, dst2)
    # unique unordered adjacent pairs of graph 1 + multi-edge merge R
    pairs = {}
    for i, (s, d) in enumerate(zip(src1, dst1)):
        pairs.setdefault((int(s), int(d)), []).append(i)
    plist = sorted(pairs)
    NU = len(plist)
    NUPAD = (NU + 7) // 8 * 8
    deg2 = [1 + len(x) for x in nbrs2]
    SPAD = max(80, (max(deg2) + sum(deg2) // NC + 7) // 8 * 8)
    cores = _plan_assignment(nbrs2, SPAD)

    R = np.zeros((E, NUPAD), np.float32)
    for u, key in enumerate(plist):
        for i in pairs[key]:
            R[i, u] = 1.0

    bf = ml_dtypes.bfloat16
    # host-precontracted integer tables (exact in bf16)
    S2R = S2 @ R
    D2R = D2M @ R
    S2H = S2 @ H1.T
    D2H = D2M @ H1.T
    ID72 = np.eye(72, dtype=np.float32)

    W72 = 2 * D + 2 * NUPAD + 2 * 72 + 2 * SPAD + 72
    W64 = 2 * D + 2 * 72
    PK64 = np.zeros((64, W64), bf)
    PK64[:, 0:D] = l1.T.astype(bf)
    PK64[:, D:2 * D] = l2.T.astype(bf)
    PK64[:, 2 * D:2 * D + 72] = U1.T.astype(bf)
    PK64[:, 2 * D + 72:2 * D + 144] = U2.T.astype(bf)
    PK72B = np.zeros((72, W72), bf)
    o = 0
    offs = {}
    for nm, w in [("f1", D), ("f2", D), ("s2r", NUPAD), ("d2r", NUPAD),
                  ("s2h", 72), ("d2h", 72), ("hs", SPAD), ("ohss", SPAD),
                  ("id72", 72)]:
        offs[nm] = o
        o += w

    def put(pk, nm, arr):
        pk[:, offs[nm]:offs[nm] + arr.shape[1]] = arr.astype(bf)
    put(PK72B, "f1", F1)
    put(PK72B, "f2", F2)
    put(PK72B, "s2r", S2R)
    put(PK72B, "d2r", D2R)
    put(PK72B, "s2h", S2H)
    put(PK72B, "d2h", D2H)
    put(PK72B, "id72", ID72)

    in_maps = []
    slot_maps = []
    for c in range(NC):
        slots = []
        for a in cores[c]:
            slots.append((a, a))
            for cc in sorted(nbrs2[a]):
                slots.append((a, cc))
        SELT = np.zeros((E, SPAD), np.float32)
        OHSS = np.zeros((72, SPAD), np.float32)
        for s_i, (a, cc) in enumerate(slots):
            SELT[:, s_i] = H2[a] * H2[cc]
            if a == cc:
                OHSS[a, s_i] = 1.0
        pk72 = PK72B.copy()
        put(pk72, "hs", H1 @ SELT)
        put(pk72, "ohss", OHSS)
        in_maps.append({"PK72": pk72, "PK64": PK64})
        slot_maps.append(slots)

    # host assembly maps: value columns + flat offsets within a block
    col_idx = np.concatenate([np.arange(72),
                              np.repeat(72 + np.arange(NU), 2)])
    offs = [b * (N * N + 1) for b in range(72)]
    for (b, d) in plist:
        offs.append(b * N * N + d)
        offs.append(d * N * N + b)
    offs_all = np.array(offs, np.int64)
    return in_maps, slot_maps, col_idx, offs_all, SPAD, NUPAD


_CACHE = {}


def kernel(**inputs):
    from concourse.bass_utils import run_bass_kernel_spmd

    in_maps, slot_maps, col_idx, offs_all, SPAD, NUPAD = _prepare(inputs)
    key = (SPAD, NUPAD)
    nc = _CACHE.get(key)
    if nc is None:
        nc = _build_nc(SPAD, NUPAD)
        _CACHE[key] = nc
    res = run_bass_kernel_spmd(nc, in_maps, list(range(NC)))
    M = np.zeros((N * N, N * N), np.float32)
    for c in range(NC):
        out = res.results[c]["OUT"]
        slots = slot_maps[c]
        bases = np.array([a * (N * N * N) + cc * N for a, cc in slots],
                         np.int64)
        M.flat[bases[:, None] + offs_all[None, :]] = \
            out[:len(slots)][:, col_idx]
    return M


# revision 10
# speedup vs baseline: 1.1948x; 1.0083x over previous
"""Trainium2 Bass kernel for nn_Affinity (gnn_message_passing).

M[(a,b),(c,d)] = sum_{j,i} H2[a,j]H2[c,j] H1[b,i]H1[d,i] W[j,i] + diag(Mp).

Structure exploited:
 - Nonzero blocks (a,c) of M: a==c or (a,c) an edge of graph 2 -> "slots".
   626 slots total, balanced 9 bands/core across 8 cores (<=79 slots/core).
 - Within a block, only graph-1-adjacent (b,d) positions (and the diagonal)
   are nonzero; blocks are symmetric in (b,d), so each block is fully
   described by 72 diagonal values + one value per unique adjacent pair
   (275 of them) -> device output is [80 slots, 352] per core.
 - Per-slot weights factor through ZS = Xsum^T SELT (the edge-affinity
   matrix Me is never materialized), and the block values are
   OUTD = ZS^T (vv H1^T), OUTO = ZS^T (vv R) where R merges multi-edges
   of graph 1 into unique pairs. diag(Mp) folds in as an extra PSUM
   accumulation against an identity table.

All index-derived tables (incidence, SELT, R, H1T, OHSS, I72) are
host-built 0/1 matrices; every floating-point op runs on device. Host
assembly only places computed values (and zeros) into the [5184, 5184]
output.
"""
import sys
sys.path.insert(0, '/opt/trn_rl_repo')
import numpy as np

N = 72
E = 288
D = 64
NC = 8


def _split_waits(nc, limit=1):
    """This walrus build rejects instructions with >limit sem waits; move the
    excess onto same-engine NoOps inserted immediately before (same bb order =
    same engine program order, so semantics are preserved)."""
    import concourse.mybir as mybir
    for f in nc.m.functions:
        for bb in f.blocks:
            new_insts = []
            for inst in bb.instructions:
                si = inst.sync_info
                waits = list(si.on_wait) if si and si.on_wait else []
                if len(waits) > limit:
                    extra, keep = waits[:-limit], waits[-limit:]
                    for i in range(0, len(extra), limit):
                        nop = mybir.InstNoOp(
                            name=nc.get_next_instruction_name(),
                            engine=inst.engine, ins=[], outs=[],
                            sync_info=mybir.SyncInfo(
                                on_wait=extra[i:i + limit], on_update=[]),
                        )
                        nc.register_instruction(nop)
                        new_insts.append(nop)
                    si.on_wait = keep
                new_insts.append(inst)
            bb.instructions[:] = new_insts


def _incidence(src, dst):
    H = np.zeros((N, E), np.float32)
    H[src, np.arange(E)] = 1.0
    H[dst, np.arange(E)] = 1.0
    return H


def _neighbors(src, dst):
    nbrs = [set() for _ in range(N)]
    for s, d in zip(src, dst):
        nbrs[int(s)].add(int(d))
        nbrs[int(d)].add(int(s))
    return nbrs


def _plan_assignment(nbrs2, spad):
    """9 bands per core, greedily balancing slot count (1 + deg per band)."""
    deg = [len(x) for x in nbrs2]
    order = sorted(range(N), key=lambda a: -deg[a])
    cores = [[] for _ in range(NC)]
    loads = [0] * NC
    for a in order:
        c = min((c for c in range(NC) if len(cores[c]) < 9),
                key=lambda c: loads[c])
        cores[c].append(a)
        loads[c] += 1 + deg[a]
    assert max(loads) <= spad
    return cores


def _build_nc(SPAD, NUPAD):
    import concourse.bass as bass
    import concourse.mybir as mybir
    import concourse.tile as tile

    F32 = mybir.dt.float32
    BF16 = mybir.dt.bfloat16
    OUTW = 72 + NUPAD
    CW = NUPAD + 72      # combo width [s2r|s2h] / [d2r|d2h]

    nc = bass.Bass()
    WA = 2 * D + 2 * CW          # f1 f2 [s2r|s2h] [d2r|d2h]
    WB = SPAD + SPAD + 72        # hs ohss id72
    W64 = 2 * D + 2 * 72         # l1t l2t u1t u2t
    pka_d = nc.declare_dram_parameter("PKA", [72, WA], BF16, isOutput=False)
    pkb_d = nc.declare_dram_parameter("PKB", [72, WB], BF16, isOutput=False)
    pk64_d = nc.declare_dram_parameter("PK64", [64, W64], BF16, isOutput=False)
    out_d = nc.declare_dram_parameter("OUT", [SPAD, OUTW], F32, isOutput=True)

    with tile.TileContext(nc) as tc:
        with tc.tile_pool(name="cst", bufs=1) as cst, \
             tc.tile_pool(name="ps", bufs=4, space="PSUM") as ps, \
             tc.tile_pool(name="psb", bufs=4, space="PSUM") as psb:

            pka = cst.tile([72, WA], BF16)
            pkb = cst.tile([72, WB], BF16)
            p64 = cst.tile([64, W64], BF16)
            nc.sync.dma_start(out=pka[:], in_=pka_d[:])
            nc.scalar.dma_start(out=p64[:], in_=pk64_d[:])
            nc.scalar.dma_start(out=pkb[:], in_=pkb_d[:])

            f1 = pka[:, 0:D]
            f2 = pka[:, D:2 * D]
            sr = pka[:, 2 * D:2 * D + CW]            # [s2r|s2h]
            dr = pka[:, 2 * D + CW:2 * D + 2 * CW]   # [d2r|d2h]
            hs = pkb[:, 0:SPAD]
            ohss = pkb[:, SPAD:2 * SPAD]
            id72 = pkb[:, 2 * SPAD:2 * SPAD + 72]
            l1t = p64[:, 0:D]
            l2t = p64[:, D:2 * D]
            u1t = p64[:, 2 * D:2 * D + 72]
            u2t = p64[:, 2 * D + 72:2 * D + 144]

            # stacked relu(lamda^T): rows 0:64 = relu(L1^T), 64:128 = relu(L2^T)
            rcat = cst.tile([128, D], BF16, tag="rcat")
            nc.vector.tensor_relu(out=rcat[0:D, :], in_=l1t)
            nc.vector.tensor_relu(out=rcat[D:2 * D, :], in_=l2t)

            # --- PE wave 1: contract F1/F2/U against host tables.
            c1_p = ps.tile([D, CW], F32, tag="mm")
            nc.tensor.matmul(out=c1_p[:], lhsT=f2, rhs=sr, start=True, stop=True)
            c2_p = ps.tile([D, CW], F32, tag="mm")
            nc.tensor.matmul(out=c2_p[:], lhsT=f2, rhs=dr, start=True, stop=True)
            zs_p = ps.tile([D, SPAD], F32, tag="mm")
            nc.tensor.matmul(out=zs_p[:], lhsT=f1, rhs=hs, start=True, stop=True)
            mp_p = ps.tile([72, 72], F32, tag="mm")
            nc.tensor.matmul(out=mp_p[:], lhsT=u1t, rhs=u2t, start=True, stop=True)

            # stacked copies: rows 0:64 <- c1, 64:128 <- c2
            fcat = cst.tile([128, CW], BF16, tag="fcat")
            nc.scalar.copy(out=fcat[0:D, :], in_=c1_p[:])
            nc.vector.tensor_copy(out=fcat[D:2 * D, :], in_=c2_p[:])
            zsc = cst.tile([D, SPAD], BF16, tag="zsc")
            nc.scalar.copy(out=zsc[:], in_=zs_p[:])
            mpc = cst.tile([72, 72], BF16, tag="mpc")
            nc.vector.tensor_copy(out=mpc[:], in_=mp_p[:])

            # --- PE wave 2: one fused lamda-apply + mpx select.
            vvk_p = psb.tile([D, CW], F32, tag="bb")
            nc.tensor.matmul(out=vvk_p[:], lhsT=rcat[:], rhs=fcat[:],
                             start=True, stop=True)
            mpx_p = ps.tile([72, SPAD], F32, tag="mm")
            nc.tensor.matmul(out=mpx_p[:], lhsT=mpc[:], rhs=ohss,
                             start=True, stop=True)

            vvkc = cst.tile([D, CW], BF16, tag="vvkc")
            nc.vector.tensor_copy(out=vvkc[:], in_=vvk_p[:])
            mpxc = cst.tile([72, SPAD], BF16, tag="mpxc")
            nc.scalar.copy(out=mpxc[:], in_=mpx_p[:])

            # --- PE wave 3: OUT[s, 72:] = ZS^T vvm;  OUT[s, :72] = ZS^T vkd
            # + Mp diag.  Two PSUM tiles so the big half ships first.
            bpB = psb.tile([SPAD, NUPAD], F32, tag="bb")
            nc.tensor.matmul(out=bpB[:], lhsT=zsc[:], rhs=vvkc[:, 0:NUPAD],
                             start=True, stop=True)
            bpA = psb.tile([SPAD, 72], F32, tag="bb")
            nc.tensor.matmul(out=bpA[:], lhsT=zsc[:], rhs=vvkc[:, NUPAD:CW],
                             start=True, stop=False)
            nc.tensor.matmul(out=bpA[:], lhsT=mpxc[:], rhs=id72,
                             start=False, stop=True)

            stgB = cst.tile([SPAD, NUPAD], F32)
            nc.scalar.copy(out=stgB[:], in_=bpB[:])
            nc.sync.dma_start(out=out_d[:, 72:OUTW], in_=stgB[:])
            stgA = cst.tile([SPAD, 72], F32)
            nc.vector.tensor_copy(out=stgA[:], in_=bpA[:])
            nc.scalar.dma_start(out=out_d[:, 0:72], in_=stgA[:])

    _split_waits(nc)
    return nc


def _prepare(inputs):
    import ml_dtypes
    ins = {k: np.asarray(v) for k, v in inputs.items()}
    F1 = ins["F1"].astype(np.float32)
    F2 = ins["F2"].astype(np.float32)
    U1 = ins["U1"].astype(np.float32)
    U2 = ins["U2"].astype(np.float32)
    l1 = ins["lamda1"].astype(np.float32)
    l2 = ins["lamda2"].astype(np.float32)
    src1 = ins["src1"].astype(np.int64)
    dst1 = ins["dst1"].astype(np.int64)
    src2 = ins["src2"].astype(np.int64)
    dst2 = ins["dst2"].astype(np.int64)

    H1 = _incidence(src1, dst1)
    H2 = _incidence(src2, dst2)
    S2 = np.zeros((N, E), np.float32)
    S2[src2, np.arange(E)] = 1.0
    D2M = np.zeros((N, E), np.float32)
    D2M[dst2, np.arange(E)] = 1.0

    nbrs2 = _neighbors(src2, dst2)
    # unique unordered adjacent pairs of graph 1 + multi-edge merge R
    pairs = {}
    for i, (s, d) in enumerate(zip(src1, dst1)):
        pairs.setdefault((int(s), int(d)), []).append(i)
    plist = sorted(pairs)
    NU = len(plist)
    NUPAD = (NU + 7) // 8 * 8
    deg2 = [1 + len(x) for x in nbrs2]
    SPAD = max(80, (max(deg2) + sum(deg2) // NC + 7) // 8 * 8)
    cores = _plan_assignment(nbrs2, SPAD)

    R = np.zeros((E, NUPAD), np.float32)
    for u, key in enumerate(plist):
        for i in pairs[key]:
            R[i, u] = 1.0

    bf = ml_dtypes.bfloat16
    # host-precontracted integer tables (exact in bf16)
    S2R = S2 @ R
    D2R = D2M @ R
    S2H = S2 @ H1.T
    D2H = D2M @ H1.T
    ID72 = np.eye(72, dtype=np.float32)

    CW = NUPAD + 72
    WA = 2 * D + 2 * CW
    WB = SPAD + SPAD + 72
    W64 = 2 * D + 2 * 72
    PK64 = np.zeros((64, W64), bf)
    PK64[:, 0:D] = l1.T.astype(bf)
    PK64[:, D:2 * D] = l2.T.astype(bf)
    PK64[:, 2 * D:2 * D + 72] = U1.T.astype(bf)
    PK64[:, 2 * D + 72:2 * D + 144] = U2.T.astype(bf)
    PKA = np.zeros((72, WA), bf)
    PKA[:, 0:D] = F1.astype(bf)
    PKA[:, D:2 * D] = F2.astype(bf)
    PKA[:, 2 * D:2 * D + NUPAD] = S2R.astype(bf)
    PKA[:, 2 * D + NUPAD:2 * D + CW] = S2H.astype(bf)
    PKA[:, 2 * D + CW:2 * D + CW + NUPAD] = D2R.astype(bf)
    PKA[:, 2 * D + CW + NUPAD:2 * D + 2 * CW] = D2H.astype(bf)

    in_maps = []
    slot_maps = []
    for c in range(NC):
        slots = []
        for a in cores[c]:
            slots.append((a, a))
            for cc in sorted(nbrs2[a]):
                slots.append((a, cc))
        SELT = np.zeros((E, SPAD), np.float32)
        OHSS = np.zeros((72, SPAD), np.float32)
        for s_i, (a, cc) in enumerate(slots):
            SELT[:, s_i] = H2[a] * H2[cc]
            if a == cc:
                OHSS[a, s_i] = 1.0
        PKB = np.zeros((72, WB), bf)
        PKB[:, 0:SPAD] = (H1 @ SELT).astype(bf)
        PKB[:, SPAD:2 * SPAD] = OHSS.astype(bf)
        PKB[:, 2 * SPAD:2 * SPAD + 72] = ID72.astype(bf)
        in_maps.append({"PKA": PKA, "PKB": PKB, "PK64": PK64})
        slot_maps.append(slots)

    # host assembly maps: value columns + flat offsets within a block
    col_idx = np.concatenate([np.arange(72),
                              np.repeat(72 + np.arange(NU), 2)])
    offs = [b * (N * N + 1) for b in range(72)]
    for (b, d) in plist:
        offs.append(b * N * N + d)
        offs.append(d * N * N + b)
    offs_all = np.array(offs, np.int64)
    return in_maps, slot_maps, col_idx, offs_all, SPAD, NUPAD


_CACHE = {}


def kernel(**inputs):
    from concourse.bass_utils import run_bass_kernel_spmd

    in_maps, slot_maps, col_idx, offs_all, SPAD, NUPAD = _prepare(inputs)
    key = (SPAD, NUPAD)
    nc = _CACHE.get(key)
    if nc is None:
        nc = _build_nc(SPAD, NUPAD)
        _CACHE[key] = nc
    res = run_bass_kernel_spmd(nc, in_maps, list(range(NC)))
    M = np.zeros((N * N, N * N), np.float32)
    for c in range(NC):
        out = res.results[c]["OUT"]
        slots = slot_maps[c]
        bases = np.array([a * (N * N * N) + cc * N for a, cc in slots],
                         np.int64)
        M.flat[bases[:, None] + offs_all[None, :]] = \
            out[:len(slots)][:, col_idx]
    return M
